# revision 35
# baseline (speedup 1.0000x reference)
"""BandSplit kernel for Trainium2 (8 NeuronCores, SPMD data-parallel).

Math: the (deterministic) melbank partitions the 1025 STFT bins into 257
contiguous segments (widths 1/4/8/8/1), all mel weights are 1.0, so

    out[b,c,t,k,o] = sum_{f in seg(k)} sum_i x[b,c,t,f,i]*pre_w[i,f,o] + pre_b[k,o]

Sharding: data-parallel over the 8 (b,c) pairs, one per core.
Per core: 256 tokens; out (256, 257, 128) -> memory bound.

Device strategy (v3, bf16 I/O + 4-way PE tiling): inputs packed to bf16
on host (~2.3 MB reads/core); per-band segment matmuls on the PE as
block-diagonal rhs with a bias ones-row, every matmul confined to ONE
32-row PE row-group (K = 9 or 17) at partition offsets cycling
0/32/64/96 -> 4 matmuls run concurrently in the 128x128 array, which
doubles cold-clock PE throughput (this part keeps the PE off the
critical path even at the throttled 1.2 GHz this platform pins).
bf16 in / fp32 PSUM accumulate; PSUM -> SBUF copies cast fp32 -> bf16
at FD=1024 (2 PSUM banks) alternating VectorE/ScalarE; output streams
out in ~512 KB chunks (2 copies per DMA) as bf16 (16.8 MB/core instead
of 33.7) and is cast back to fp32 on host. Output DMAs issue only from
sync (HWDGE) and gpsimd (SWDGE, first ~40us only, so its ~10us
dge_drain postamble hides under the copy phase) - never from the copy
engines. Total HBM traffic per core ~19 MB vs 38.7 for fp32.
Rel err ~5e-3 (bf16 rounding), inside the 2e-2 gate.
"""

import numpy as np
import ml_dtypes

import concourse.bacc as bacc
import concourse.mybir as mybir
from concourse.tile import TileContext
from concourse.bass_utils import run_bass_kernel_spmd

BF16 = np.dtype(ml_dtypes.bfloat16)

# ---------------------------------------------------------------- structure

B, C, T, NF, IN_CH = 4, 2, 256, 1025, 2
N_BANDS, OUT_CH = 257, 128
N_CORES = 8
TOK = 256           # tokens per core (= T; one (b,c) pair per core)
HALVES = 2          # 128-token tiles


def _segments():
    segs = []
    for k in range(N_BANDS):
        if k < 128:
            segs.append((k, 1))
        elif k < 160:
            segs.append((128 + 4 * (k - 128), 4))
        elif k < 192:
            segs.append((256 + 8 * (k - 160), 8))
        elif k < 256:
            segs.append((512 + 8 * (k - 192), 8))
        else:
            segs.append((1024, 1))
    return segs


SEGS = _segments()


def _build_plan():
    """Matmul descriptors. Every mm fits one 32-row PE row-group:
    class A (width-1 bands): 4 bands, K=9, N=512;
    class B (width-4 bands): 2 bands, K=17, N=256;
    class C (width-8 bands): 1 band,  K=17, N=128.
    off cycles 0/32/64/96 so 4 consecutive mms run concurrently."""
    plan = []
    for a in range(33):  # A: bands 0..127 (+ band 256 runt at idx 32)
        bands = [256] if a == 32 else list(range(4 * a, 4 * a + 4))
        plan.append(dict(
            bands=bands, g=a // 4, off=32 * (a % 4),
            K=sum(2 * SEGS[k][1] for k in bands) + 1,
            N=128 * len(bands),
            wcol=512 * (a // 4) if a < 32 else 4096,
        ))
    for b in range(16):  # B: bands 128..159, two per mm
        bands = [128 + 2 * b, 128 + 2 * b + 1]
        plan.append(dict(
            bands=bands, g=9 + b // 4, off=32 * (b % 4),
            K=17, N=256,
            wcol=4224 + 256 * (b // 4),
        ))
    for c in range(96):  # C: bands 160..255, one per mm
        plan.append(dict(
            bands=[160 + c], g=13 + c // 4, off=32 * (c % 4),
            K=17, N=128,
            wcol=5248 + 128 * (c // 4),
        ))
    return plan


PLAN = _build_plan()
NG = 37                      # x column groups
XCOLS = NG * TOK             # 9472
WCOLS = 5248 + 3072          # 8320

# Output chunks (~512 KB, one DMA each): 2 PSUM tiles of 1024 cols (one
# fp32->bf16 copy each, drained by different engines in parallel) filled
# by matmuls emitted round-robin over the four 32-row PE row-groups.
# PSUM-bank safety: two matmuls may run CONCURRENTLY on the PE iff their
# row-groups differ, and concurrent matmuls must never write the same
# 2 KB PSUM bank (hardware collision -> kernel abort). The placement
# below gives the 4 mms of every concurrency window 4 distinct banks;
# same-row-group mms (which serialize on the PE) share banks safely.
# chunk = dict(tiles=[[mm,...] per tile in column order],
#              emit=[(mm, tile_idx, col), ...] in emission order)


def _build_chunks():
    chunks = []
    for k in range(8):     # A quartets: 4 mms N=512, one bank each
        ms = [4 * k + j for j in range(4)]
        ch = dict(
            tiles=[[ms[0], ms[2]], [ms[1], ms[3]]],
            emit=[(ms[0], 0, 0), (ms[1], 1, 0),
                  (ms[2], 0, 512), (ms[3], 1, 512)],
        )
        if k == 7:         # band-256 runt rides with the last A chunk
            ch["tiles"].append([32])
            ch["emit"].append((32, 2, 0))
        chunks.append(ch)
    for q in range(2):     # B: 8 mms N=256 per chunk
        m0 = 33 + 8 * q
        tiles = [[m0, m0 + 4, m0 + 1, m0 + 5],
                 [m0 + 2, m0 + 6, m0 + 3, m0 + 7]]
        emit = [(m0 + j, (j % 4) // 2, 512 * (j % 2) + 256 * (j // 4))
                for j in range(8)]
        chunks.append(dict(tiles=tiles, emit=emit))
    for o in range(6):     # C: 16 mms N=128 per chunk
        m0 = 49 + 16 * o
        tiles = [[m0 + 4 * i + t for i in range(4)] +
                 [m0 + 4 * i + t + 1 for i in range(4)]
                 for t in (0, 2)]
        emit = [(m0 + j, (j % 4) // 2, 512 * (j % 2) + 128 * (j // 4))
                for j in range(16)]
        chunks.append(dict(tiles=tiles, emit=emit))
    return chunks


_CHUNKS = _build_chunks()

# (h, chunk, flat elem offset, ntot) in emission order.
_OCHUNKS = []
_o = 0
for _h in range(HALVES):
    for _ch in _CHUNKS:
        ntot = sum(PLAN[i]["N"] for t in _ch["tiles"] for i in t)
        _OCHUNKS.append((_h, _ch, _o, ntot))
        _o += 128 * ntot
OELEMS = _o  # == TOK * N_BANDS * OUT_CH

# load regions (row_lo, row_hi, col_lo, col_hi). Row-sliced loads
# concentrate descriptors on the few SDMA engines serving those
# partitions, so only the SMALL class-A regions (first matmuls, ~0.5 MB
# total) are sliced; the big B/C regions load full-width (zero padding
# included) and stream at line rate on all 16 engines — a 17-row B/C
# slice would trickle for ~10 us and head-of-line-block output chunks
# queued behind it on the same engine rings.
_XLOADS = [
    (0, 9, 0, 2304),
    (32, 41, 0, 2304),
    (64, 73, 0, 2304),
    (96, 105, 0, 2304),
    (0, 128, 2304, XCOLS),
]
_WLOADS = [
    (0, 9, 0, 4224),
    (32, 41, 0, 4096),
    (64, 73, 0, 4096),
    (96, 105, 0, 4096),
    (0, 128, 4224, WCOLS),
]


def _xmm_index():
    """Fancy-index arrays to build x_mm from xt (2050, TOK)."""
    src, dstg, dstr, og, orow = [], [], [], [], []
    for mm in PLAN:
        r = 0
        for k in mm["bands"]:
            f0, w = SEGS[k]
            for l in range(w):
                for i in range(IN_CH):
                    src.append((f0 + l) * 2 + i)
                    dstg.append(mm["g"])
                    dstr.append(mm["off"] + r)
                    r += 1
        og.append(mm["g"])
        orow.append(mm["off"] + r)
    return (np.array(src), np.array(dstg), np.array(dstr),
            np.array(og), np.array(orow))


_XSRC, _XDG, _XDR, _XOG, _XOR = _xmm_index()

# ---------------------------------------------------------------- host prep


def _build_wmm(pre_w, pre_b):
    """(128, WCOLS) bf16: per-mm block-diagonal weights + bias ones-row."""
    wmm = np.zeros((128, WCOLS), dtype=np.float32)
    for mm in PLAN:
        off, wc = mm["off"], mm["wcol"]
        r = 0
        for j, k in enumerate(mm["bands"]):
            f0, w = SEGS[k]
            cols = slice(wc + 128 * j, wc + 128 * (j + 1))
            for l in range(w):
                for i in range(IN_CH):
                    wmm[off + r, cols] = pre_w[i, f0 + l, :]
                    r += 1
            wmm[off + mm["K"] - 1, cols] = pre_b[k, :]
    return wmm.astype(BF16)


def _build_xmm(x_core):
    """x_core (TOK, NF, IN_CH) -> (128, XCOLS) bf16 packed lhsT layout."""
    xt = np.ascontiguousarray(x_core.reshape(TOK, NF * IN_CH).T)  # (2050, TOK)
    xmm = np.zeros((NG, 128, TOK), dtype=np.float32)
    xmm[_XDG, _XDR, :] = xt[_XSRC, :]
    xmm[_XOG, _XOR, :] = 1.0
    return np.ascontiguousarray(
        xmm.transpose(1, 0, 2)).reshape(128, XCOLS).astype(BF16)


def _assemble(out_flat):
    """flat device output (bf16) -> (TOK, N_BANDS, OUT_CH) fp32."""
    oc = np.empty((TOK, N_BANDS, OUT_CH), dtype=np.float32)
    for h, ch, o, ntot in _OCHUNKS:
        blk = out_flat[o:o + 128 * ntot].reshape(128, ntot)
        c = 0
        for t in ch["tiles"]:
            for i in t:
                mm = PLAN[i]
                nb = len(mm["bands"])
                k0 = mm["bands"][0]
                oc[h * 128:(h + 1) * 128, k0:k0 + nb, :] = (
                    blk[:, c:c + mm["N"]].reshape(128, nb, OUT_CH))
                c += mm["N"]
    return oc


# ---------------------------------------------------------------- device

_PROGRAM = None


def _build_program():
    global _PROGRAM
    if _PROGRAM is not None:
        return _PROGRAM

    nc = bacc.Bacc("TRN2", target_bir_lowering=False)
    f32 = mybir.dt.float32
    bf16 = mybir.dt.bfloat16
    xin = nc.dram_tensor("xmm", [128, XCOLS], bf16, kind="ExternalInput")
    win = nc.dram_tensor("wmm", [128, WCOLS], bf16, kind="ExternalInput")
    out = nc.dram_tensor("out", [OELEMS], bf16, kind="ExternalOutput")

    with TileContext(nc) as tc:
        with (
            tc.tile_pool(name="xw", bufs=1) as xw_pool,
            tc.tile_pool(name="stage", bufs=6) as stage_pool,
            tc.tile_pool(name="psum", bufs=4, space="PSUM") as psum_pool,
        ):
            x_sb = xw_pool.tile([128, XCOLS], bf16, tag="x")
            w_sb = xw_pool.tile([128, WCOLS], bf16, tag="w")
            for r0, r1, c0, c1 in _XLOADS:
                nc.sync.dma_start(out=x_sb[r0:r1, c0:c1],
                                  in_=xin.ap()[r0:r1, c0:c1])
            for r0, r1, c0, c1 in _WLOADS:
                nc.scalar.dma_start(out=w_sb[r0:r1, c0:c1],
                                    in_=win.ap()[r0:r1, c0:c1])

            def emit_mm(ps, pc, i, tcol):
                mm = PLAN[i]
                off, K, N = mm["off"], mm["K"], mm["N"]
                gcol = mm["g"] * TOK + tcol
                nc.tensor.matmul(
                    ps[:, pc:pc + N],
                    x_sb[off:off + K, gcol:gcol + 128],
                    w_sb[off:off + K, mm["wcol"]:mm["wcol"] + N],
                    start=True, stop=True,
                    tile_position=(off, 0),
                )
                return pc + N

            # Output DMA issue map: never from the copy engines (a
            # DMA_DIRECT2D on scalar/vector stretches the copy stream
            # 1:1). gpsimd/SWDGE takes the first chunks of each half so
            # all its issues land before ~40us and its ~10us dge_drain
            # postamble hides under the copy phase; sync takes the rest.
            nchunks = len(_CHUNKS)

            # all output on sync (one HWDGE queue sustains ~407 GB/s when
            # fed; adding queues cannot beat the ~430 GB/s HBM cap), except
            # the last 4 chunks, which drain on the by-then-idle scalar
            # queue so the end-of-kernel backlog empties on two queues.
            def ring_for(h, ci):
                if h == 1 and ci >= nchunks - 4:
                    return nc.scalar
                return nc.sync

            oc_it = iter(_OCHUNKS)
            for h in range(HALVES):
                for ci in range(nchunks):
                    (_, ch, o, ntot) = next(oc_it)
                    tcol = h * 128
                    tiles = ch["tiles"]
                    sb = stage_pool.tile([128, ntot], bf16, tag="st",
                                         name="sb")
                    # one PSUM tile per chunk tile; matmuls emitted in
                    # ch["emit"] order (round-robin over the 4 row-groups,
                    # bank-safe placement)
                    pts = [psum_pool.tile([128, 1024], f32, tag="ps",
                                          name="pt") for _ in tiles]
                    sizes = [sum(PLAN[i]["N"] for i in t) for t in tiles]
                    for (i, ti, col) in ch["emit"]:
                        emit_mm(pts[ti], col, i, tcol)
                    # copies: tile0 -> E0, tile1 -> E1 in parallel
                    # (runt tile -> E0); roles swap in half 1
                    c = 0
                    for ti, tile in enumerate(tiles):
                        use_dve = (ti != 1) == (h == 0)
                        dst = sb[:, c:c + sizes[ti]]
                        if use_dve:
                            nc.vector.tensor_copy(dst, pts[ti][:, 0:sizes[ti]])
                        else:
                            nc.scalar.copy(dst, pts[ti][:, 0:sizes[ti]])
                        c += sizes[ti]
                    ring_for(h, ci).dma_start(
                        out=out.ap()[o:o + 128 * ntot]
                            .rearrange("(p n) -> p n", n=ntot),
                        in_=sb[:],
                    )

    nc.compile()
    _PROGRAM = nc
    return nc


# ---------------------------------------------------------------- entry

LAST_RESULTS = None  # BassKernelResults of the most recent run (for test.py)


def kernel(x, pre_w, pre_b, _trace=False):
    global LAST_RESULTS
    x = np.asarray(x, dtype=np.float32)
    pre_w = np.asarray(pre_w, dtype=np.float32)
    pre_b = np.asarray(pre_b, dtype=np.float32)
    assert x.shape == (B, C, T, NF, IN_CH), x.shape

    nc = _build_program()
    wmm = _build_wmm(pre_w, pre_b)
    in_maps = []
    for core in range(N_CORES):
        b_, c_ = divmod(core, C)
        in_maps.append({"xmm": _build_xmm(x[b_, c_]), "wmm": wmm})

    res = run_bass_kernel_spmd(
        nc, in_maps, core_ids=list(range(N_CORES)), trace=_trace,
    )
    LAST_RESULTS = res

    out = np.empty((B, C, T, N_BANDS, OUT_CH), dtype=np.float32)
    for core in range(N_CORES):
        b_, c_ = divmod(core, C)
        out[b_, c_] = _assemble(res.results[core]["out"])
    return out


# revision 37
# speedup vs baseline: 1.0126x; 1.0126x over previous
"""BandSplit kernel for Trainium2 (8 NeuronCores, SPMD data-parallel).

Math: the (deterministic) melbank partitions the 1025 STFT bins into 257
contiguous segments (widths 1/4/8/8/1), all mel weights are 1.0, so

    out[b,c,t,k,o] = sum_{f in seg(k)} sum_i x[b,c,t,f,i]*pre_w[i,f,o] + pre_b[k,o]

Sharding: data-parallel over the 8 (b,c) pairs, one per core.
Per core: 256 tokens; out (256, 257, 128) -> memory bound.

Device strategy (v3, bf16 I/O + 4-way PE tiling): inputs packed to bf16
on host (~2.3 MB reads/core); per-band segment matmuls on the PE as
block-diagonal rhs with a bias ones-row, every matmul confined to ONE
32-row PE row-group (K = 9 or 17) at partition offsets cycling
0/32/64/96 -> 4 matmuls run concurrently in the 128x128 array, which
doubles cold-clock PE throughput (this part keeps the PE off the
critical path even at the throttled 1.2 GHz this platform pins).
bf16 in / fp32 PSUM accumulate; PSUM -> SBUF copies cast fp32 -> bf16
at FD=1024 (2 PSUM banks) alternating VectorE/ScalarE; output streams
out in ~512 KB chunks (2 copies per DMA) as bf16 (16.8 MB/core instead
of 33.7) and is cast back to fp32 on host. Output DMAs issue only from
sync (HWDGE) and gpsimd (SWDGE, first ~40us only, so its ~10us
dge_drain postamble hides under the copy phase) - never from the copy
engines. Total HBM traffic per core ~19 MB vs 38.7 for fp32.
Rel err ~5e-3 (bf16 rounding), inside the 2e-2 gate.
"""

import numpy as np
import ml_dtypes

import concourse.bacc as bacc
import concourse.mybir as mybir
from concourse.tile import TileContext
from concourse.bass_utils import run_bass_kernel_spmd

BF16 = np.dtype(ml_dtypes.bfloat16)

# ---------------------------------------------------------------- structure

B, C, T, NF, IN_CH = 4, 2, 256, 1025, 2
N_BANDS, OUT_CH = 257, 128
N_CORES = 8
TOK = 256           # tokens per core (= T; one (b,c) pair per core)
HALVES = 2          # 128-token tiles


def _segments():
    segs = []
    for k in range(N_BANDS):
        if k < 128:
            segs.append((k, 1))
        elif k < 160:
            segs.append((128 + 4 * (k - 128), 4))
        elif k < 192:
            segs.append((256 + 8 * (k - 160), 8))
        elif k < 256:
            segs.append((512 + 8 * (k - 192), 8))
        else:
            segs.append((1024, 1))
    return segs


SEGS = _segments()


def _build_plan():
    """Matmul descriptors. Every mm fits one 32-row PE row-group:
    class A (width-1 bands): 4 bands, K=9, N=512;
    class B (width-4 bands): 2 bands, K=17, N=256;
    class C (width-8 bands): 1 band,  K=17, N=128.
    off cycles 0/32/64/96 so 4 consecutive mms run concurrently."""
    plan = []
    for a in range(33):  # A: bands 0..127 (+ band 256 runt at idx 32)
        bands = [256] if a == 32 else list(range(4 * a, 4 * a + 4))
        plan.append(dict(
            bands=bands, g=a // 4, off=32 * (a % 4),
            K=sum(2 * SEGS[k][1] for k in bands) + 1,
            N=128 * len(bands),
            wcol=512 * (a // 4) if a < 32 else 4096,
        ))
    for b in range(16):  # B: bands 128..159, two per mm
        bands = [128 + 2 * b, 128 + 2 * b + 1]
        plan.append(dict(
            bands=bands, g=9 + b // 4, off=32 * (b % 4),
            K=17, N=256,
            wcol=4224 + 256 * (b // 4),
        ))
    for c in range(96):  # C: bands 160..255, one per mm
        plan.append(dict(
            bands=[160 + c], g=13 + c // 4, off=32 * (c % 4),
            K=17, N=128,
            wcol=5248 + 128 * (c // 4),
        ))
    return plan


PLAN = _build_plan()
NG = 37                      # x column groups
XCOLS = NG * TOK             # 9472
WCOLS = 5248 + 3072          # 8320

# Output chunks (~512 KB, one DMA each): 2 PSUM tiles of 1024 cols (one
# fp32->bf16 copy each, drained by different engines in parallel) filled
# by matmuls emitted round-robin over the four 32-row PE row-groups.
# PSUM-bank safety: two matmuls may run CONCURRENTLY on the PE iff their
# row-groups differ, and concurrent matmuls must never write the same
# 2 KB PSUM bank (hardware collision -> kernel abort). The placement
# below gives the 4 mms of every concurrency window 4 distinct banks;
# same-row-group mms (which serialize on the PE) share banks safely.
# chunk = dict(tiles=[[mm,...] per tile in column order],
#              emit=[(mm, tile_idx, col), ...] in emission order)


def _build_chunks():
    chunks = []
    for k in range(8):     # A quartets: 4 mms N=512, one bank each
        ms = [4 * k + j for j in range(4)]
        ch = dict(
            tiles=[[ms[0], ms[2]], [ms[1], ms[3]]],
            emit=[(ms[0], 0, 0), (ms[1], 1, 0),
                  (ms[2], 0, 512), (ms[3], 1, 512)],
        )
        if k == 7:         # band-256 runt rides with the last A chunk
            ch["tiles"].append([32])
            ch["emit"].append((32, 2, 0))
        chunks.append(ch)
    for q in range(2):     # B: 8 mms N=256 per chunk
        m0 = 33 + 8 * q
        tiles = [[m0, m0 + 4, m0 + 1, m0 + 5],
                 [m0 + 2, m0 + 6, m0 + 3, m0 + 7]]
        emit = [(m0 + j, (j % 4) // 2, 512 * (j % 2) + 256 * (j // 4))
                for j in range(8)]
        chunks.append(dict(tiles=tiles, emit=emit))
    for o in range(6):     # C: 16 mms N=128 per chunk
        m0 = 49 + 16 * o
        tiles = [[m0 + 4 * i + t for i in range(4)] +
                 [m0 + 4 * i + t + 1 for i in range(4)]
                 for t in (0, 2)]
        emit = [(m0 + j, (j % 4) // 2, 512 * (j % 2) + 128 * (j // 4))
                for j in range(16)]
        chunks.append(dict(tiles=tiles, emit=emit))
    return chunks


_CHUNKS = _build_chunks()

# (h, chunk, flat elem offset, ntot) in emission order.
_OCHUNKS = []
_o = 0
for _h in range(HALVES):
    for _ch in _CHUNKS:
        ntot = sum(PLAN[i]["N"] for t in _ch["tiles"] for i in t)
        _OCHUNKS.append((_h, _ch, _o, ntot))
        _o += 128 * ntot
OELEMS = _o  # == TOK * N_BANDS * OUT_CH

# full 128-partition load regions (row_lo, row_hi, col_lo, col_hi).
# Row-sliced loads (9/17 rows) concentrate their descriptors on the few
# SDMA engines serving those partitions, making the loads trickle for
# ~10+ us and head-of-line-blocking output chunks queued behind them on
# the same engine rings. Loading the full 128 rows (zero padding
# included, ~+2 MB) streams at line rate on all 16 engines. The first
# quartet's columns load separately so the first matmul starts ~5 us
# earlier.
_XLOADS = [
    (0, 128, 0, 256),        # quartet 0
    (0, 128, 256, 2304),     # rest of class A
    (0, 128, 2304, XCOLS),   # B/C
]
_WLOADS = [
    (0, 128, 0, 512),
    (0, 128, 512, 4224),
    (0, 128, 4224, WCOLS),
]


def _xmm_index():
    """Fancy-index arrays to build x_mm from xt (2050, TOK)."""
    src, dstg, dstr, og, orow = [], [], [], [], []
    for mm in PLAN:
        r = 0
        for k in mm["bands"]:
            f0, w = SEGS[k]
            for l in range(w):
                for i in range(IN_CH):
                    src.append((f0 + l) * 2 + i)
                    dstg.append(mm["g"])
                    dstr.append(mm["off"] + r)
                    r += 1
        og.append(mm["g"])
        orow.append(mm["off"] + r)
    return (np.array(src), np.array(dstg), np.array(dstr),
            np.array(og), np.array(orow))


_XSRC, _XDG, _XDR, _XOG, _XOR = _xmm_index()

# ---------------------------------------------------------------- host prep


def _build_wmm(pre_w, pre_b):
    """(128, WCOLS) bf16: per-mm block-diagonal weights + bias ones-row."""
    wmm = np.zeros((128, WCOLS), dtype=np.float32)
    for mm in PLAN:
        off, wc = mm["off"], mm["wcol"]
        r = 0
        for j, k in enumerate(mm["bands"]):
            f0, w = SEGS[k]
            cols = slice(wc + 128 * j, wc + 128 * (j + 1))
            for l in range(w):
                for i in range(IN_CH):
                    wmm[off + r, cols] = pre_w[i, f0 + l, :]
                    r += 1
            wmm[off + mm["K"] - 1, cols] = pre_b[k, :]
    return wmm.astype(BF16)


def _build_xmm(x_core):
    """x_core (TOK, NF, IN_CH) -> (128, XCOLS) bf16 packed lhsT layout."""
    xt = np.ascontiguousarray(x_core.reshape(TOK, NF * IN_CH).T)  # (2050, TOK)
    xmm = np.zeros((NG, 128, TOK), dtype=np.float32)
    xmm[_XDG, _XDR, :] = xt[_XSRC, :]
    xmm[_XOG, _XOR, :] = 1.0
    return np.ascontiguousarray(
        xmm.transpose(1, 0, 2)).reshape(128, XCOLS).astype(BF16)


def _assemble(out_flat):
    """flat device output (bf16) -> (TOK, N_BANDS, OUT_CH) fp32."""
    oc = np.empty((TOK, N_BANDS, OUT_CH), dtype=np.float32)
    for h, ch, o, ntot in _OCHUNKS:
        blk = out_flat[o:o + 128 * ntot].reshape(128, ntot)
        c = 0
        for t in ch["tiles"]:
            for i in t:
                mm = PLAN[i]
                nb = len(mm["bands"])
                k0 = mm["bands"][0]
                oc[h * 128:(h + 1) * 128, k0:k0 + nb, :] = (
                    blk[:, c:c + mm["N"]].reshape(128, nb, OUT_CH))
                c += mm["N"]
    return oc


# ---------------------------------------------------------------- device

_PROGRAM = None


def _build_program():
    global _PROGRAM
    if _PROGRAM is not None:
        return _PROGRAM

    nc = bacc.Bacc("TRN2", target_bir_lowering=False)
    f32 = mybir.dt.float32
    bf16 = mybir.dt.bfloat16
    xin = nc.dram_tensor("xmm", [128, XCOLS], bf16, kind="ExternalInput")
    win = nc.dram_tensor("wmm", [128, WCOLS], bf16, kind="ExternalInput")
    out = nc.dram_tensor("out", [OELEMS], bf16, kind="ExternalOutput")

    with TileContext(nc) as tc:
        with (
            tc.tile_pool(name="xw", bufs=1) as xw_pool,
            tc.tile_pool(name="stage", bufs=6) as stage_pool,
            tc.tile_pool(name="psum", bufs=4, space="PSUM") as psum_pool,
        ):
            x_sb = xw_pool.tile([128, XCOLS], bf16, tag="x")
            w_sb = xw_pool.tile([128, WCOLS], bf16, tag="w")
            for r0, r1, c0, c1 in _XLOADS:
                nc.sync.dma_start(out=x_sb[r0:r1, c0:c1],
                                  in_=xin.ap()[r0:r1, c0:c1])
            for r0, r1, c0, c1 in _WLOADS:
                nc.scalar.dma_start(out=w_sb[r0:r1, c0:c1],
                                    in_=win.ap()[r0:r1, c0:c1])

            def emit_mm(ps, pc, i, tcol):
                mm = PLAN[i]
                off, K, N = mm["off"], mm["K"], mm["N"]
                gcol = mm["g"] * TOK + tcol
                nc.tensor.matmul(
                    ps[:, pc:pc + N],
                    x_sb[off:off + K, gcol:gcol + 128],
                    w_sb[off:off + K, mm["wcol"]:mm["wcol"] + N],
                    start=True, stop=True,
                    tile_position=(off, 0),
                )
                return pc + N

            # Output DMA issue map: never from the copy engines (a
            # DMA_DIRECT2D on scalar/vector stretches the copy stream
            # 1:1). gpsimd/SWDGE takes the first chunks of each half so
            # all its issues land before ~40us and its ~10us dge_drain
            # postamble hides under the copy phase; sync takes the rest.
            nchunks = len(_CHUNKS)

            # all output on sync: one HWDGE queue sustains ~407 GB/s when
            # fed, and adding queues cannot beat the ~430 GB/s HBM cap.
            def ring_for(h, ci):
                return nc.sync

            oc_it = iter(_OCHUNKS)
            for h in range(HALVES):
                for ci in range(nchunks):
                    (_, ch, o, ntot) = next(oc_it)
                    tcol = h * 128
                    tiles = ch["tiles"]
                    sb = stage_pool.tile([128, ntot], bf16, tag="st",
                                         name="sb")
                    # one PSUM tile per chunk tile; matmuls emitted in
                    # ch["emit"] order (round-robin over the 4 row-groups,
                    # bank-safe placement)
                    pts = [psum_pool.tile([128, 1024], f32, tag="ps",
                                          name="pt") for _ in tiles]
                    sizes = [sum(PLAN[i]["N"] for i in t) for t in tiles]
                    for (i, ti, col) in ch["emit"]:
                        emit_mm(pts[ti], col, i, tcol)
                    # copies: tile0 -> E0, tile1 -> E1 in parallel
                    # (runt tile -> E0); roles swap in half 1
                    c = 0
                    for ti, tile in enumerate(tiles):
                        use_dve = (ti != 1) == (h == 0)
                        dst = sb[:, c:c + sizes[ti]]
                        if use_dve:
                            nc.vector.tensor_copy(dst, pts[ti][:, 0:sizes[ti]])
                        else:
                            nc.scalar.copy(dst, pts[ti][:, 0:sizes[ti]])
                        c += sizes[ti]
                    ring_for(h, ci).dma_start(
                        out=out.ap()[o:o + 128 * ntot]
                            .rearrange("(p n) -> p n", n=ntot),
                        in_=sb[:],
                    )

    nc.compile()
    _PROGRAM = nc
    return nc


# ---------------------------------------------------------------- entry

LAST_RESULTS = None  # BassKernelResults of the most recent run (for test.py)


def kernel(x, pre_w, pre_b, _trace=False):
    global LAST_RESULTS
    x = np.asarray(x, dtype=np.float32)
    pre_w = np.asarray(pre_w, dtype=np.float32)
    pre_b = np.asarray(pre_b, dtype=np.float32)
    assert x.shape == (B, C, T, NF, IN_CH), x.shape

    nc = _build_program()
    wmm = _build_wmm(pre_w, pre_b)
    in_maps = []
    for core in range(N_CORES):
        b_, c_ = divmod(core, C)
        in_maps.append({"xmm": _build_xmm(x[b_, c_]), "wmm": wmm})

    res = run_bass_kernel_spmd(
        nc, in_maps, core_ids=list(range(N_CORES)), trace=_trace,
    )
    LAST_RESULTS = res

    out = np.empty((B, C, T, N_BANDS, OUT_CH), dtype=np.float32)
    for core in range(N_CORES):
        b_, c_ = divmod(core, C)
        out[b_, c_] = _assemble(res.results[core]["out"])
    return out


# revision 38
# speedup vs baseline: 1.1323x; 1.1182x over previous
"""BandSplit kernel for Trainium2 (8 NeuronCores, SPMD data-parallel).

Math: the (deterministic) melbank partitions the 1025 STFT bins into 257
contiguous segments (widths 1/4/8/8/1), all mel weights are 1.0, so

    out[b,c,t,k,o] = sum_{f in seg(k)} sum_i x[b,c,t,f,i]*pre_w[i,f,o] + pre_b[k,o]

Sharding: data-parallel over the 8 (b,c) pairs, one per core.
Per core: 256 tokens; out (256, 257, 128) -> memory bound.

Device strategy (v3, bf16 I/O + 4-way PE tiling): inputs packed to bf16
on host (~2.3 MB reads/core); per-band segment matmuls on the PE as
block-diagonal rhs with a bias ones-row, every matmul confined to ONE
32-row PE row-group (K = 9 or 17) at partition offsets cycling
0/32/64/96 -> 4 matmuls run concurrently in the 128x128 array, which
doubles cold-clock PE throughput (this part keeps the PE off the
critical path even at the throttled 1.2 GHz this platform pins).
bf16 in / fp32 PSUM accumulate; PSUM -> SBUF copies cast fp32 -> bf16
at FD=1024 (2 PSUM banks) alternating VectorE/ScalarE; output streams
out in ~512 KB chunks (2 copies per DMA) as bf16 (16.8 MB/core instead
of 33.7) and is cast back to fp32 on host. Output DMAs issue only from
sync (HWDGE) and gpsimd (SWDGE, first ~40us only, so its ~10us
dge_drain postamble hides under the copy phase) - never from the copy
engines. Total HBM traffic per core ~19 MB vs 38.7 for fp32.
Rel err ~5e-3 (bf16 rounding), inside the 2e-2 gate.
"""

import numpy as np
import ml_dtypes

import concourse.bacc as bacc
import concourse.mybir as mybir
from concourse.tile import TileContext
from concourse.bass_utils import run_bass_kernel_spmd

BF16 = np.dtype(ml_dtypes.bfloat16)

# ---------------------------------------------------------------- structure

B, C, T, NF, IN_CH = 4, 2, 256, 1025, 2
N_BANDS, OUT_CH = 257, 128
N_CORES = 8
TOK = 256           # tokens per core (= T; one (b,c) pair per core)
HALVES = 2          # 128-token tiles


def _segments():
    segs = []
    for k in range(N_BANDS):
        if k < 128:
            segs.append((k, 1))
        elif k < 160:
            segs.append((128 + 4 * (k - 128), 4))
        elif k < 192:
            segs.append((256 + 8 * (k - 160), 8))
        elif k < 256:
            segs.append((512 + 8 * (k - 192), 8))
        else:
            segs.append((1024, 1))
    return segs


SEGS = _segments()


def _build_plan():
    """Matmul descriptors. Every mm fits one 32-row PE row-group:
    class A (width-1 bands): 4 bands, K=9, N=512;
    class B (width-4 bands): 2 bands, K=17, N=256;
    class C (width-8 bands): 1 band,  K=17, N=128.
    off cycles 0/32/64/96 so 4 consecutive mms run concurrently."""
    plan = []
    for a in range(33):  # A: bands 0..127 (+ band 256 runt at idx 32)
        bands = [256] if a == 32 else list(range(4 * a, 4 * a + 4))
        plan.append(dict(
            bands=bands, g=a // 4, off=32 * (a % 4),
            K=sum(2 * SEGS[k][1] for k in bands) + 1,
            N=128 * len(bands),
            wcol=512 * (a // 4) if a < 32 else 4096,
        ))
    for b in range(16):  # B: bands 128..159, two per mm
        bands = [128 + 2 * b, 128 + 2 * b + 1]
        plan.append(dict(
            bands=bands, g=9 + b // 4, off=32 * (b % 4),
            K=17, N=256,
            wcol=4224 + 256 * (b // 4),
        ))
    for c in range(96):  # C: bands 160..255, one per mm
        plan.append(dict(
            bands=[160 + c], g=13 + c // 4, off=32 * (c % 4),
            K=17, N=128,
            wcol=5248 + 128 * (c // 4),
        ))
    return plan


PLAN = _build_plan()
NG = 37                      # x column groups
XCOLS = NG * TOK             # 9472
WCOLS = 5248 + 3072          # 8320

# Output chunks (~512 KB, one DMA each): 2 PSUM tiles of 1024 cols (one
# fp32->bf16 copy each, drained by different engines in parallel) filled
# by matmuls emitted round-robin over the four 32-row PE row-groups.
# PSUM-bank safety: two matmuls may run CONCURRENTLY on the PE iff their
# row-groups differ, and concurrent matmuls must never write the same
# 2 KB PSUM bank (hardware collision -> kernel abort). The placement
# below gives the 4 mms of every concurrency window 4 distinct banks;
# same-row-group mms (which serialize on the PE) share banks safely.
# chunk = dict(tiles=[[mm,...] per tile in column order],
#              emit=[(mm, tile_idx, col), ...] in emission order)


def _build_chunks():
    chunks = []
    for k in range(8):     # A quartets: 4 mms N=512, one bank each
        ms = [4 * k + j for j in range(4)]
        ch = dict(
            tiles=[[ms[0], ms[2]], [ms[1], ms[3]]],
            emit=[(ms[0], 0, 0), (ms[1], 1, 0),
                  (ms[2], 0, 512), (ms[3], 1, 512)],
        )
        if k == 7:         # band-256 runt rides with the last A chunk
            ch["tiles"].append([32])
            ch["emit"].append((32, 2, 0))
        chunks.append(ch)
    for q in range(2):     # B: 8 mms N=256 per chunk
        m0 = 33 + 8 * q
        tiles = [[m0, m0 + 4, m0 + 1, m0 + 5],
                 [m0 + 2, m0 + 6, m0 + 3, m0 + 7]]
        emit = [(m0 + j, (j % 4) // 2, 512 * (j % 2) + 256 * (j // 4))
                for j in range(8)]
        chunks.append(dict(tiles=tiles, emit=emit))
    for o in range(6):     # C: 16 mms N=128 per chunk
        m0 = 49 + 16 * o
        tiles = [[m0 + 4 * i + t for i in range(4)] +
                 [m0 + 4 * i + t + 1 for i in range(4)]
                 for t in (0, 2)]
        emit = [(m0 + j, (j % 4) // 2, 512 * (j % 2) + 128 * (j // 4))
                for j in range(16)]
        chunks.append(dict(tiles=tiles, emit=emit))
    return chunks


_CHUNKS = _build_chunks()

# (h, chunk, flat elem offset, ntot) in emission order.
_OCHUNKS = []
_o = 0
for _h in range(HALVES):
    for _ch in _CHUNKS:
        ntot = sum(PLAN[i]["N"] for t in _ch["tiles"] for i in t)
        _OCHUNKS.append((_h, _ch, _o, ntot))
        _o += 128 * ntot
OELEMS = _o  # == TOK * N_BANDS * OUT_CH

# full 128-partition load regions (row_lo, row_hi, col_lo, col_hi).
# Row-sliced loads (9/17 rows) concentrate their descriptors on the few
# SDMA engines serving those partitions, making the loads trickle for
# ~10+ us and head-of-line-blocking output chunks queued behind them on
# the same engine rings. Loading the full 128 rows (zero padding
# included, ~+2 MB) streams at line rate on all 16 engines. The first
# quartet's columns load separately so the first matmul starts ~5 us
# earlier.
_XLOADS = [
    (0, 128, 0, 2304),       # class A columns (first matmuls)
    (0, 128, 2304, XCOLS),   # B/C columns
]
_WLOADS = [
    (0, 128, 0, 4224),
    (0, 128, 4224, WCOLS),
]


def _xmm_index():
    """Fancy-index arrays to build x_mm from xt (2050, TOK)."""
    src, dstg, dstr, og, orow = [], [], [], [], []
    for mm in PLAN:
        r = 0
        for k in mm["bands"]:
            f0, w = SEGS[k]
            for l in range(w):
                for i in range(IN_CH):
                    src.append((f0 + l) * 2 + i)
                    dstg.append(mm["g"])
                    dstr.append(mm["off"] + r)
                    r += 1
        og.append(mm["g"])
        orow.append(mm["off"] + r)
    return (np.array(src), np.array(dstg), np.array(dstr),
            np.array(og), np.array(orow))


_XSRC, _XDG, _XDR, _XOG, _XOR = _xmm_index()

# ---------------------------------------------------------------- host prep


def _build_wmm(pre_w, pre_b):
    """(128, WCOLS) bf16: per-mm block-diagonal weights + bias ones-row."""
    wmm = np.zeros((128, WCOLS), dtype=np.float32)
    for mm in PLAN:
        off, wc = mm["off"], mm["wcol"]
        r = 0
        for j, k in enumerate(mm["bands"]):
            f0, w = SEGS[k]
            cols = slice(wc + 128 * j, wc + 128 * (j + 1))
            for l in range(w):
                for i in range(IN_CH):
                    wmm[off + r, cols] = pre_w[i, f0 + l, :]
                    r += 1
            wmm[off + mm["K"] - 1, cols] = pre_b[k, :]
    return wmm.astype(BF16)


def _build_xmm(x_core):
    """x_core (TOK, NF, IN_CH) -> (128, XCOLS) bf16 packed lhsT layout."""
    xt = np.ascontiguousarray(x_core.reshape(TOK, NF * IN_CH).T)  # (2050, TOK)
    xmm = np.zeros((NG, 128, TOK), dtype=np.float32)
    xmm[_XDG, _XDR, :] = xt[_XSRC, :]
    xmm[_XOG, _XOR, :] = 1.0
    return np.ascontiguousarray(
        xmm.transpose(1, 0, 2)).reshape(128, XCOLS).astype(BF16)


def _assemble(out_flat):
    """flat device output (bf16) -> (TOK, N_BANDS, OUT_CH) fp32."""
    oc = np.empty((TOK, N_BANDS, OUT_CH), dtype=np.float32)
    for h, ch, o, ntot in _OCHUNKS:
        blk = out_flat[o:o + 128 * ntot].reshape(128, ntot)
        c = 0
        for t in ch["tiles"]:
            for i in t:
                mm = PLAN[i]
                nb = len(mm["bands"])
                k0 = mm["bands"][0]
                oc[h * 128:(h + 1) * 128, k0:k0 + nb, :] = (
                    blk[:, c:c + mm["N"]].reshape(128, nb, OUT_CH))
                c += mm["N"]
    return oc


# ---------------------------------------------------------------- device

_PROGRAM = None


def _build_program():
    global _PROGRAM
    if _PROGRAM is not None:
        return _PROGRAM

    nc = bacc.Bacc("TRN2", target_bir_lowering=False)
    f32 = mybir.dt.float32
    bf16 = mybir.dt.bfloat16
    xin = nc.dram_tensor("xmm", [128, XCOLS], bf16, kind="ExternalInput")
    win = nc.dram_tensor("wmm", [128, WCOLS], bf16, kind="ExternalInput")
    out = nc.dram_tensor("out", [OELEMS], bf16, kind="ExternalOutput")

    with TileContext(nc) as tc:
        with (
            tc.tile_pool(name="xw", bufs=1) as xw_pool,
            tc.tile_pool(name="stage", bufs=6) as stage_pool,
            tc.tile_pool(name="psum", bufs=4, space="PSUM") as psum_pool,
        ):
            x_sb = xw_pool.tile([128, XCOLS], bf16, tag="x")
            w_sb = xw_pool.tile([128, WCOLS], bf16, tag="w")
            for r0, r1, c0, c1 in _XLOADS:
                nc.sync.dma_start(out=x_sb[r0:r1, c0:c1],
                                  in_=xin.ap()[r0:r1, c0:c1])
            for r0, r1, c0, c1 in _WLOADS:
                nc.scalar.dma_start(out=w_sb[r0:r1, c0:c1],
                                    in_=win.ap()[r0:r1, c0:c1])

            def emit_mm(ps, pc, i, tcol):
                mm = PLAN[i]
                off, K, N = mm["off"], mm["K"], mm["N"]
                gcol = mm["g"] * TOK + tcol
                nc.tensor.matmul(
                    ps[:, pc:pc + N],
                    x_sb[off:off + K, gcol:gcol + 128],
                    w_sb[off:off + K, mm["wcol"]:mm["wcol"] + N],
                    start=True, stop=True,
                    tile_position=(off, 0),
                )
                return pc + N

            # Output DMA issue map: never from the copy engines (a
            # DMA_DIRECT2D on scalar/vector stretches the copy stream
            # 1:1). gpsimd/SWDGE takes the first chunks of each half so
            # all its issues land before ~40us and its ~10us dge_drain
            # postamble hides under the copy phase; sync takes the rest.
            nchunks = len(_CHUNKS)

            # all output on sync: one HWDGE queue sustains ~407 GB/s when
            # fed, and adding queues cannot beat the ~430 GB/s HBM cap.
            def ring_for(h, ci):
                return nc.sync

            oc_it = iter(_OCHUNKS)
            for h in range(HALVES):
                for ci in range(nchunks):
                    (_, ch, o, ntot) = next(oc_it)
                    tcol = h * 128
                    tiles = ch["tiles"]
                    sb = stage_pool.tile([128, ntot], bf16, tag="st",
                                         name="sb")
                    # one PSUM tile per chunk tile; matmuls emitted in
                    # ch["emit"] order (round-robin over the 4 row-groups,
                    # bank-safe placement)
                    pts = [psum_pool.tile([128, 1024], f32, tag="ps",
                                          name="pt") for _ in tiles]
                    sizes = [sum(PLAN[i]["N"] for i in t) for t in tiles]
                    for (i, ti, col) in ch["emit"]:
                        emit_mm(pts[ti], col, i, tcol)
                    # copies: tile0 -> E0, tile1 -> E1 in parallel
                    # (runt tile -> E0); roles swap in half 1
                    c = 0
                    for ti, tile in enumerate(tiles):
                        use_dve = (ti != 1) == (h == 0)
                        dst = sb[:, c:c + sizes[ti]]
                        if use_dve:
                            nc.vector.tensor_copy(dst, pts[ti][:, 0:sizes[ti]])
                        else:
                            nc.scalar.copy(dst, pts[ti][:, 0:sizes[ti]])
                        c += sizes[ti]
                    ring_for(h, ci).dma_start(
                        out=out.ap()[o:o + 128 * ntot]
                            .rearrange("(p n) -> p n", n=ntot),
                        in_=sb[:],
                    )

    nc.compile()
    _PROGRAM = nc
    return nc


# ---------------------------------------------------------------- entry

LAST_RESULTS = None  # BassKernelResults of the most recent run (for test.py)


def kernel(x, pre_w, pre_b, _trace=False):
    global LAST_RESULTS
    x = np.asarray(x, dtype=np.float32)
    pre_w = np.asarray(pre_w, dtype=np.float32)
    pre_b = np.asarray(pre_b, dtype=np.float32)
    assert x.shape == (B, C, T, NF, IN_CH), x.shape

    nc = _build_program()
    wmm = _build_wmm(pre_w, pre_b)
    in_maps = []
    for core in range(N_CORES):
        b_, c_ = divmod(core, C)
        in_maps.append({"xmm": _build_xmm(x[b_, c_]), "wmm": wmm})

    res = run_bass_kernel_spmd(
        nc, in_maps, core_ids=list(range(N_CORES)), trace=_trace,
    )
    LAST_RESULTS = res

    out = np.empty((B, C, T, N_BANDS, OUT_CH), dtype=np.float32)
    for core in range(N_CORES):
        b_, c_ = divmod(core, C)
        out[b_, c_] = _assemble(res.results[core]["out"])
    return out
